# revision 1
# baseline (speedup 1.0000x reference)
"""Trainium2 Bass kernel for nn_Attention_44994077393310.

Multi-head attention (B=8, N=2048, C=768, H=4, Dh=192) with input projections,
softmax attention, and output projection with bias.

Sharding: pure data-parallel over the batch dim - each of the 8 NeuronCores
computes one batch element end-to-end (weights replicated). No collectives.

Layout strategy: q/k/v and all weight matrices are pre-transposed ON THE HOST
(cheap numpy work that is not device time), so the device kernel never
transposes anything: every DMA lands operands exactly where the PE wants them
(contraction dim on partitions).

Per-core dataflow:
  - qT/kT/vT [c, n] and WqT/WkT/WvT/WpT [c, j] stream in via SWDGE cast-DMA
    (fp32 DRAM -> float32r SBUF; fp32r = fp32 with 12 mantissa bits dropped,
    ~2.4e-4 rounding, 4x the matmul throughput of fp32).
  - k/v projections produce khT [c', n] head-major (a-tile dd 0..127, packed
    b-tiles dd 128..191 of two heads) and vh natural [n, (h, dd + ones-col)];
    the ones column makes the softmax denominators fall out of the same
    matmuls that compute U = attn_unnorm @ v.
  - scores are computed TRANSPOSED: S_T[kk, qq], exp on ScalarE with the
    1/sqrt(dh) scale folded in; U_T accumulates over the 16 k-tiles.
  - per-query 1/rowsum is broadcast across partitions with a rank-1 ones
    matmul; normalization happens in the PSUM evacuation multiply. The head
    finalization is software-pipelined into the NEXT head's score loop so the
    slow 1-partition RECIPROCAL never stalls the PE.
  - final projection consumes xT as the stationary operand so y comes out
    NATURAL [n, j]; bias is added during PSUM evacuation from a
    partition-broadcast bias tile.
"""

import numpy as np

B = 8
N = 2048
C = 768
H = 4
DH = 192
SCALE = DH ** -0.5

NCHUNKS = 4                # chunks of 512 over the sequence
CHUNK = N // NCHUNKS       # 512
CC = C // 128              # 6 channel chunks
KT = N // 128              # 16 k-tiles
JGW = 384                  # j-group width for natural-output projections
NJG = C // JGW             # 2

_MM_F32R = True

_BUILT = None


def _dest_of(cp):
    h, dd = divmod(cp, DH)
    if dd < 128:
        return ("a", h, dd)
    return ("b", h // 2, (h % 2) * 64 + (dd - 128))


def _jc_segments(jc):
    """Merged PSUM->head-major copy segments for projection j-chunk jc."""
    segs = []
    for p0 in range(0, 128, 64):
        kind, idx, dlo = _dest_of(128 * jc + p0)
        if segs and segs[-1][2] == kind and segs[-1][3] == idx and \
                segs[-1][4] + (segs[-1][1] - segs[-1][0]) == dlo:
            segs[-1] = (segs[-1][0], p0 + 64, kind, idx, segs[-1][4])
        else:
            segs.append((p0, p0 + 64, kind, idx, dlo))
    return segs


def _build(mm_f32r=_MM_F32R):
    from contextlib import ExitStack

    import concourse.mybir as mybir
    import concourse.tile as tile
    from concourse import bacc

    F32 = mybir.dt.float32
    MMD = mybir.dt.float32r if mm_f32r else F32
    AF = mybir.ActivationFunctionType

    nc = bacc.Bacc("TRN2", target_bir_lowering=False, debug=False)
    qt_d = nc.dram_tensor("qT", [C, N], F32, kind="ExternalInput").ap()
    kt_d = nc.dram_tensor("kT", [C, N], F32, kind="ExternalInput").ap()
    vt_d = nc.dram_tensor("vT", [C, N], F32, kind="ExternalInput").ap()
    wqt_d = nc.dram_tensor("WqT", [C, C], F32, kind="ExternalInput").ap()
    wkt_d = nc.dram_tensor("WkT", [C, C], F32, kind="ExternalInput").ap()
    wvt_d = nc.dram_tensor("WvT", [C, C], F32, kind="ExternalInput").ap()
    wpt_d = nc.dram_tensor("WpT", [C, C], F32, kind="ExternalInput").ap()
    bp_d = nc.dram_tensor("bp", [C], F32, kind="ExternalInput").ap()
    y_d = nc.dram_tensor("y", [N, C], F32, kind="ExternalOutput").ap()

    with tile.TileContext(nc) as tc, ExitStack() as ctx:
        const = ctx.enter_context(tc.tile_pool(name="const", bufs=1))
        wqp = ctx.enter_context(tc.tile_pool(name="wqp", bufs=1))
        khp = ctx.enter_context(tc.tile_pool(name="khp", bufs=1))
        vhp = ctx.enter_context(tc.tile_pool(name="vhp", bufs=1))
        xtp = ctx.enter_context(tc.tile_pool(name="xT", bufs=2))
        psA = ctx.enter_context(tc.tile_pool(name="psA", bufs=2, space="PSUM"))
        psP = ctx.enter_context(tc.tile_pool(name="psP", bufs=2, space="PSUM"))
        psUa = ctx.enter_context(tc.tile_pool(name="psUa", bufs=2, space="PSUM"))
        psUb = ctx.enter_context(tc.tile_pool(name="psUb", bufs=2, space="PSUM"))

        ones_col_f32 = const.tile([128, H], F32, tag="ones_col", name="ones_col")
        nc.vector.memset(ones_col_f32[:], 1.0)
        ones_row_f32 = const.tile([1, 128], F32, tag="ones_row_f", name="ones_row_f")
        nc.vector.memset(ones_row_f32[:], 1.0)
        ones_row = const.tile([1, 128], MMD, tag="ones_row", name="ones_row")
        nc.vector.tensor_copy(ones_row[:], ones_row_f32[:])

        # PE warm-up: dependency-free matmuls so the HAM clock gate opens
        # while the first DMAs stream in.
        warm_w_f = const.tile([128, 128], F32, tag="warm_w_f", name="warm_w_f")
        nc.vector.memset(warm_w_f[:], 0.5)
        warm_w = const.tile([128, 128], MMD, tag="warm_w", name="warm_w")
        nc.vector.tensor_copy(warm_w[:], warm_w_f[:])
        warm_x = const.tile([128, 512], MMD, tag="warm_x", name="warm_x")
        for i in range(4):
            nc.vector.tensor_copy(warm_x[:, i * 128:(i + 1) * 128], warm_w_f[:])
        for r in range(64):
            wp = psUa.tile([128, 512], F32, tag="psUa", name="psUa")
            nc.tensor.matmul(wp[:], warm_w[:], warm_x[:], start=True, stop=True)

        # ---- persistent weights (direct cast-DMA loads, no transposes) ----
        WqT = wqp.tile([128, CC, C], MMD, tag="wqt", name="wqt")
        WpT_a = wqp.tile([128, H, C], MMD, tag="wpa", name="wpa")
        WpT_b = [wqp.tile([128, C], MMD, tag=f"wpb{g}", name=f"wpb{g}")
                 for g in range(2)]
        bias_bc = wqp.tile([128, C], F32, tag="bias_bc", name="bias_bc")

        khT_a = [khp.tile([128, N], MMD, tag=f"kha{h}", name=f"kha{h}")
                 for h in range(H)]
        khT_b = [khp.tile([128, N], MMD, tag=f"khb{g}", name=f"khb{g}")
                 for g in range(2)]
        vh = [vhp.tile([128, H, DH + 1], MMD, tag=f"vh{nt}", name=f"vh{nt}")
              for nt in range(KT)]

        def load_wT_grouped(dest, w_dram):
            # dest[p, cc, j] = W.T[cc*128+p, j]
            nc.gpsimd.dma_start(
                dest[:],
                w_dram.rearrange("(cc p) j -> p cc j", p=128))

        def seg_dest(kind, idx, dlo, dhi, a_tiles, b_tiles, col_lo, col_hi):
            t = a_tiles[idx] if kind == "a" else b_tiles[idx]
            return t[dlo:dhi, col_lo:col_hi]

        # ---- phase 1: stage k, v --------------------------------------
        with tc.tile_pool(name="wkv", bufs=1) as wkv:
            WkT = wkv.tile([128, CC, C], MMD, tag="wkt", name="wkt")
            WvT = wkv.tile([128, CC, C], MMD, tag="wvt", name="wvt")
            load_wT_grouped(WkT, wkt_d)
            load_wT_grouped(WvT, wvt_d)

            def load_wq():
                load_wT_grouped(WqT, wqt_d)

            def load_wp_bias():
                # wpt_d is host-packed head-major: rows 0..511 = per-head
                # dd 0..127 (h-major), rows 512..639 / 640..767 = the packed
                # b-tiles (dd 128..191 of heads 0,1 / 2,3).
                nc.gpsimd.dma_start(
                    WpT_a[:],
                    wpt_d[0:512, :].rearrange("(h p) j -> p h j", p=128))
                for g in range(2):
                    nc.gpsimd.dma_start(
                        WpT_b[g][:], wpt_d[512 + g * 128:512 + (g + 1) * 128, :])
                bp_row = wkv.tile([1, C], F32, tag="bp_row", name="bp_row")
                bp_row_r = wkv.tile([1, C], MMD, tag="bp_row_r", name="bp_row_r")
                nc.sync.dma_start(bp_row[:], bp_d[None, :])
                nc.vector.tensor_copy(bp_row_r[:], bp_row[:])
                for jg in range(NJG):
                    ps = psP.tile([128, 512], F32, tag="psP", name="psP")
                    nc.tensor.matmul(ps[:, 0:JGW], ones_row[:],
                                     bp_row_r[:, jg * JGW:(jg + 1) * JGW],
                                     start=True, stop=True)
                    nc.scalar.copy(bias_bc[:, jg * JGW:(jg + 1) * JGW],
                                   ps[:, 0:JGW])

            for ch in range(NCHUNKS):
                n0 = ch * CHUNK
                # -- k chunk: one batched cast-DMA, project to khT ----------
                kTt = xtp.tile([128, CC, CHUNK], MMD, tag="xT", name="kTt")
                nc.gpsimd.dma_start(
                    kTt[:],
                    kt_d[:, n0:n0 + CHUNK].rearrange("(cc p) n -> p cc n", p=128))
                for jc0 in range(0, CC, 2):
                    pss = [psP.tile([128, 512], F32, tag="psP", name="psP")
                           for _ in range(2)]
                    for cc in range(CC):
                        for i in range(2):
                            jc = jc0 + i
                            nc.tensor.matmul(
                                pss[i][:],
                                WkT[:, cc, jc * 128:(jc + 1) * 128],
                                kTt[:, cc, :], start=(cc == 0),
                                stop=(cc == CC - 1))
                    for i in range(2):
                        for (plo, phi, kind, idx, dlo) in _jc_segments(jc0 + i):
                            nc.vector.tensor_copy(
                                seg_dest(kind, idx, dlo, dlo + (phi - plo),
                                         khT_a, khT_b, n0, n0 + CHUNK),
                                pss[i][plo:phi, :])
                # -- v chunk: project to vh natural -------------------------
                vTt = xtp.tile([128, CC, CHUNK], MMD, tag="xT", name="vTt")
                nc.gpsimd.dma_start(
                    vTt[:],
                    vt_d[:, n0:n0 + CHUNK].rearrange("(cc p) n -> p cc n", p=128))
                for ntl in range(4):
                    nt = ch * 4 + ntl
                    pss = [psP.tile([128, 512], F32, tag="psP", name="psP")
                           for _ in range(NJG)]
                    for cc in range(CC):
                        for jg in range(NJG):
                            nc.tensor.matmul(
                                pss[jg][:, 0:JGW],
                                vTt[:, cc, ntl * 128:(ntl + 1) * 128],
                                WvT[:, cc, jg * JGW:(jg + 1) * JGW],
                                start=(cc == 0), stop=(cc == CC - 1))
                    for jg in range(NJG):
                        nc.vector.tensor_copy(
                            vh[nt][:, 2 * jg:2 * jg + 2, 0:DH],
                            pss[jg][:, 0:JGW].rearrange("p (h d) -> p h d", h=2))
                    nc.vector.tensor_copy(
                        vh[nt][:, :, DH:DH + 1],
                        ones_col_f32[:].rearrange("p (h o) -> p h o", h=H))
                if ch == 0:
                    load_wq()
                elif ch == 1:
                    load_wp_bias()

        # ---- phase 2: per q-chunk attention + output projection -----------
        qhp = ctx.enter_context(tc.tile_pool(name="qhp", bufs=1))
        esp = ctx.enter_context(tc.tile_pool(name="esp", bufs=3))
        xop = ctx.enter_context(tc.tile_pool(name="xop", bufs=1))
        scp = ctx.enter_context(tc.tile_pool(name="scp", bufs=2))
        yp = ctx.enter_context(tc.tile_pool(name="yp", bufs=2))

        def q_load(qc):
            n0 = qc * CHUNK
            qTt = xtp.tile([128, CC, CHUNK], MMD, tag="xT", name="qTt")
            nc.gpsimd.dma_start(
                qTt[:],
                qt_d[:, n0:n0 + CHUNK].rearrange("(cc p) n -> p cc n", p=128))
            return qTt

        def q_proj(qc, qTt):
            qhT_a = [qhp.tile([128, CHUNK], MMD, tag=f"qha{h}", name=f"qha{h}")
                     for h in range(H)]
            qhT_b = [qhp.tile([128, CHUNK], MMD, tag=f"qhb{g}", name=f"qhb{g}")
                     for g in range(2)]
            for jc0 in range(0, CC, 2):
                pss = [psP.tile([128, 512], F32, tag="psP", name="psP")
                       for _ in range(2)]
                for cc in range(CC):
                    for i in range(2):
                        jc = jc0 + i
                        nc.tensor.matmul(
                            pss[i][:],
                            WqT[:, cc, jc * 128:(jc + 1) * 128],
                            qTt[:, cc, :], start=(cc == 0), stop=(cc == CC - 1))
                for i in range(2):
                    for (plo, phi, kind, idx, dlo) in _jc_segments(jc0 + i):
                        nc.vector.tensor_copy(
                            seg_dest(kind, idx, dlo, dlo + (phi - plo),
                                     qhT_a, qhT_b, 0, CHUNK),
                            pss[i][plo:phi, :])
            return qhT_a, qhT_b

        def finalize_pre(fu_b):
            # 1-partition RECIPROCAL of the rowsum row. DVE iterative divide
            # (3.4us) - slow but off the PE; ScalarE's table Reciprocal would
            # be 6x faster but evicts the Exp table set (they cannot share the
            # bucket RAM) and the reload thrash costs far more.
            recip = scp.tile([1, CHUNK], MMD, tag="recip", name="recip",
                             bufs=1)
            with nc.allow_low_precision(reason="softmax denom recip f32r"):
                nc.vector.reciprocal(recip[:], fu_b[64:65, :])
            return recip

        def finalize_post(xT_a, xT_b, fh, fu_a, fu_b, recip):
            # broadcast 1/rowsum across partitions (rank-1 ones matmul - the
            # GpSimd partition_broadcast alternative serializes against DVE on
            # the shared SBUF port and slows the whole attention loop down)
            # and normalize during the PSUM evacuation multiplies.
            fblo = (fh % 2) * 64
            bc_ps = psA.tile([128, 512], F32, tag="psA", name="psA")
            nc.tensor.matmul(bc_ps[:], ones_row[:], recip[:],
                             start=True, stop=True)
            bc = scp.tile([128, CHUNK], F32, tag="bc", name="bc", bufs=1)
            nc.scalar.copy(bc[:], bc_ps[:])
            nc.vector.tensor_mul(xT_a[fh][:], fu_a[:], bc[:])
            nc.vector.tensor_mul(xT_b[fh // 2][fblo:fblo + 64, :],
                                 fu_b[0:64, :], bc[0:64, :])

        def attention(qc, qhT_a, qhT_b):
            xT_a = [xop.tile([128, CHUNK], MMD, tag=f"xta{h}", name=f"xta{h}")
                    for h in range(H)]
            xT_b = [xop.tile([128, CHUNK], MMD, tag=f"xtb{g}", name=f"xtb{g}")
                    for g in range(2)]

            def finalize(fh, fu_a, fu_b):
                finalize_post(xT_a, xT_b, fh, fu_a, fu_b, finalize_pre(fu_b))

            pend = None
            for h in range(H):
                blo = (h % 2) * 64
                kb = khT_b[h // 2]
                qb = qhT_b[h // 2]
                u_a = psUa.tile([128, 512], F32, tag="psUa", name="psUa")
                u_b = psUb.tile([65, 512], F32, tag="psUb", name="psUb")
                es_tiles = [None] * KT

                def scores(kt):
                    s = psA.tile([128, 512], F32, tag="psA", name="psA")
                    nc.tensor.matmul(
                        s[:], khT_a[h][:, kt * 128:(kt + 1) * 128],
                        qhT_a[h][:], start=True, stop=False)
                    nc.tensor.matmul(
                        s[:], kb[blo:blo + 64, kt * 128:(kt + 1) * 128],
                        qb[blo:blo + 64, :], start=False, stop=True)
                    es = esp.tile([128, CHUNK], MMD, tag="es", name="es")
                    nc.scalar.activation(es[:], s[:], AF.Exp, scale=SCALE)
                    es_tiles[kt] = es

                def av(kt):
                    es = es_tiles[kt]
                    nc.tensor.matmul(u_a[:], vh[kt][:, h, 0:128], es[:],
                                     start=(kt == 0), stop=(kt == KT - 1))
                    nc.tensor.matmul(u_b[:], vh[kt][:, h, 128:DH + 1], es[:],
                                     start=(kt == 0), stop=(kt == KT - 1))

                scores(0)
                for kt in range(KT - 1):
                    scores(kt + 1)
                    av(kt)
                    if kt == 4 and pend is not None:
                        finalize(*pend)
                        pend = None
                av(KT - 1)
                pend = (h, u_a, u_b)
            # last head: recip starts now; the broadcast + muls are emitted by
            # final_proj between its h0-h2 partial sums so the PE never idles
            # longer than the HAM window.
            recip = finalize_pre(pend[2])
            return xT_a, xT_b, pend, recip

        def final_proj(qc, xT_a, xT_b, pend, recip):
            n0 = qc * CHUNK

            def part_a(pss, ntl):
                # h0..h2 contributions: independent of the pending last-head
                # normalization.
                for h in range(H - 1):
                    blo = (h % 2) * 64
                    for jg in range(NJG):
                        nc.tensor.matmul(
                            pss[jg][:, 0:JGW],
                            xT_a[h][:, ntl * 128:(ntl + 1) * 128],
                            WpT_a[:, h, jg * JGW:(jg + 1) * JGW],
                            start=(h == 0), stop=False)
                    for jg in range(NJG):
                        nc.tensor.matmul(
                            pss[jg][:, 0:JGW],
                            xT_b[h // 2][blo:blo + 64, ntl * 128:(ntl + 1) * 128],
                            WpT_b[h // 2][blo:blo + 64, jg * JGW:(jg + 1) * JGW],
                            start=False, stop=False)

            def part_b(pss, ntl):
                h = H - 1
                blo = (h % 2) * 64
                for jg in range(NJG):
                    nc.tensor.matmul(
                        pss[jg][:, 0:JGW],
                        xT_a[h][:, ntl * 128:(ntl + 1) * 128],
                        WpT_a[:, h, jg * JGW:(jg + 1) * JGW],
                        start=False, stop=False)
                for jg in range(NJG):
                    nc.tensor.matmul(
                        pss[jg][:, 0:JGW],
                        xT_b[h // 2][blo:blo + 64, ntl * 128:(ntl + 1) * 128],
                        WpT_b[h // 2][blo:blo + 64, jg * JGW:(jg + 1) * JGW],
                        start=False, stop=True)

            def evac(pss, ntl):
                ysb = yp.tile([128, C], F32, tag="y", name="y")
                for jg in range(NJG):
                    nc.vector.tensor_add(ysb[:, jg * JGW:(jg + 1) * JGW],
                                         pss[jg][:, 0:JGW],
                                         bias_bc[:, jg * JGW:(jg + 1) * JGW])
                nc.sync.dma_start(
                    y_d[n0 + ntl * 128:n0 + (ntl + 1) * 128, :], ysb[:])

            # groups alternate between the psP and psUa pools (psUa is idle
            # once attention ends) so two groups stay in flight; the 24
            # h0-h2 matmuls of groups 0-1 run while the last head's
            # normalization chain drains.
            def group_tiles(ntl):
                pool, tag = (psP, "psP") if ntl % 2 == 0 else (psUa, "psUa")
                return [pool.tile([128, 512], F32, tag=tag, name=tag)
                        for _ in range(NJG)]

            g0 = group_tiles(0)
            part_a(g0, 0)
            g1 = group_tiles(1)
            part_a(g1, 1)
            finalize_post(xT_a, xT_b, *pend, recip)
            part_b(g0, 0)
            evac(g0, 0)
            part_b(g1, 1)
            evac(g1, 1)
            for ntl in range(2, 4):
                pss = group_tiles(ntl)
                part_a(pss, ntl)
                part_b(pss, ntl)
                evac(pss, ntl)

        # q-chunk pipeline: the next chunk's qT DMA streams during this
        # chunk's attention, and its projection matmuls sit between
        # attention and final_proj as ready PE work that covers the last
        # head's normalization chain.
        qh = q_proj(0, q_load(0))
        for qc in range(NCHUNKS):
            if qc + 1 < NCHUNKS:
                qt_next = q_load(qc + 1)
            xt = attention(qc, *qh)
            if qc + 1 < NCHUNKS:
                qh = q_proj(qc + 1, qt_next)
            final_proj(qc, *xt)

    nc.compile()
    return nc


def _get_built():
    global _BUILT
    if _BUILT is None:
        _BUILT = _build()
    return _BUILT


def run(inputs, trace=False, **kw):
    """Run on all 8 cores; returns (y [B,N,C] float32, BassKernelResults)."""
    from concourse.bass_utils import run_bass_kernel_spmd

    nc = _get_built()
    f32 = np.float32
    wpt = np.asarray(inputs["Wp"], f32).T  # [c', j]
    wpt_packed = np.concatenate(
        [wpt[h * DH:h * DH + 128] for h in range(H)]
        + [wpt[h * DH + 128:(h + 1) * DH] for h in range(H)])
    shared = {
        "WqT": np.ascontiguousarray(np.asarray(inputs["Wq"], f32).T),
        "WkT": np.ascontiguousarray(np.asarray(inputs["Wk"], f32).T),
        "WvT": np.ascontiguousarray(np.asarray(inputs["Wv"], f32).T),
        "WpT": np.ascontiguousarray(wpt_packed),
        "bp": np.ascontiguousarray(np.asarray(inputs["bp"], f32)),
    }
    q = np.asarray(inputs["q"], f32)
    k = np.asarray(inputs["k"], f32)
    v = np.asarray(inputs["v"], f32)
    in_maps = []
    for b in range(B):
        m = dict(shared)
        m["qT"] = np.ascontiguousarray(q[b].T)
        m["kT"] = np.ascontiguousarray(k[b].T)
        m["vT"] = np.ascontiguousarray(v[b].T)
        in_maps.append(m)
    res = run_bass_kernel_spmd(nc, in_maps, list(range(B)), trace=trace, **kw)
    y = np.stack([res.results[b]["y"] for b in range(B)]).astype(np.float32)
    return y, res


def kernel(q, k, v, Wq, Wk, Wv, Wp, bp):
    y, _ = run({"q": q, "k": k, "v": v, "Wq": Wq, "Wk": Wk, "Wv": Wv,
                "Wp": Wp, "bp": bp})
    return y



# revision 3
# speedup vs baseline: 1.2803x; 1.2803x over previous
"""Trainium2 Bass kernel for nn_Attention_44994077393310.

Multi-head attention (B=8, N=2048, C=768, H=4, Dh=192) with input projections,
softmax attention, and output projection with bias.

Sharding: pure data-parallel over the batch dim - each of the 8 NeuronCores
computes one batch element end-to-end (weights replicated). No collectives.

Layout strategy: q/k/v and all weight matrices are pre-transposed ON THE HOST
(cheap numpy work that is not device time), so the device kernel never
transposes anything: every DMA lands operands exactly where the PE wants them
(contraction dim on partitions).

Per-core dataflow:
  - qT/kT/vT [c, n] and WqT/WkT/WvT/WpT [c, j] stream in via SWDGE cast-DMA
    (fp32 DRAM -> float32r SBUF; fp32r = fp32 with 12 mantissa bits dropped,
    ~2.4e-4 rounding, 4x the matmul throughput of fp32).
  - k/v projections produce khT [c', n] head-major (a-tile dd 0..127, packed
    b-tiles dd 128..191 of two heads) and vh natural [n, (h, dd + ones-col)];
    the ones column makes the softmax denominators fall out of the same
    matmuls that compute U = attn_unnorm @ v.
  - scores are computed TRANSPOSED: S_T[kk, qq], exp on ScalarE with the
    1/sqrt(dh) scale folded in; U_T accumulates over the 16 k-tiles.
  - per-query 1/rowsum is broadcast across partitions with a rank-1 ones
    matmul; normalization happens in the PSUM evacuation multiply. The head
    finalization is software-pipelined into the NEXT head's score loop so the
    slow 1-partition RECIPROCAL never stalls the PE.
  - final projection consumes xT as the stationary operand so y comes out
    NATURAL [n, j]; bias is added during PSUM evacuation from a
    partition-broadcast bias tile.
"""

import numpy as np

B = 8
N = 2048
C = 768
H = 4
DH = 192
SCALE = DH ** -0.5

NCHUNKS = 4                # chunks of 512 over the sequence
CHUNK = N // NCHUNKS       # 512
CC = C // 128              # 6 channel chunks
KT = N // 128              # 16 k-tiles
JGW = 384                  # j-group width for natural-output projections
NJG = C // JGW             # 2

# Matmul operand dtype for SBUF tiles. fp16 (and bf16) get fast-weight-load
# (2 elems/cycle on LDWEIGHTS) which fp32/fp32r do not; with one LDWEIGHTS per
# matmul in the attention loop the fp32r weight-load path is the critical
# path (LDW ~182ns/128col vs ~90ns fp16), so 16-bit operands buy back ~2x on
# the two matmuls per k-tile that otherwise start before their weights land.
# fp16 over bf16: 10 vs 8 mantissa bits, and all dynamic range here is tame
# (scores ~N(0,1), exp<=~250, accumulation is always fp32 in PSUM).
_MM_DT = "float16"

_BUILT = None


def _dest_of(cp):
    h, dd = divmod(cp, DH)
    if dd < 128:
        return ("a", h, dd)
    return ("b", h // 2, (h % 2) * 64 + (dd - 128))


def _jc_segments(jc):
    """Merged PSUM->head-major copy segments for projection j-chunk jc."""
    segs = []
    for p0 in range(0, 128, 64):
        kind, idx, dlo = _dest_of(128 * jc + p0)
        if segs and segs[-1][2] == kind and segs[-1][3] == idx and \
                segs[-1][4] + (segs[-1][1] - segs[-1][0]) == dlo:
            segs[-1] = (segs[-1][0], p0 + 64, kind, idx, segs[-1][4])
        else:
            segs.append((p0, p0 + 64, kind, idx, dlo))
    return segs


def _build(mm_dt=_MM_DT):
    from contextlib import ExitStack

    import concourse.mybir as mybir
    import concourse.tile as tile
    from concourse import bacc

    F32 = mybir.dt.float32
    MMD = getattr(mybir.dt, mm_dt)
    AF = mybir.ActivationFunctionType

    nc = bacc.Bacc("TRN2", target_bir_lowering=False, debug=False)
    qt_d = nc.dram_tensor("qT", [C, N], F32, kind="ExternalInput").ap()
    kt_d = nc.dram_tensor("kT", [C, N], F32, kind="ExternalInput").ap()
    vt_d = nc.dram_tensor("vT", [C, N], F32, kind="ExternalInput").ap()
    wqt_d = nc.dram_tensor("WqT", [C, C], F32, kind="ExternalInput").ap()
    wkt_d = nc.dram_tensor("WkT", [C, C], F32, kind="ExternalInput").ap()
    wvt_d = nc.dram_tensor("WvT", [C, C], F32, kind="ExternalInput").ap()
    wpt_d = nc.dram_tensor("WpT", [C, C], F32, kind="ExternalInput").ap()
    bp_d = nc.dram_tensor("bp", [C], F32, kind="ExternalInput").ap()
    y_d = nc.dram_tensor("y", [N, C], F32, kind="ExternalOutput").ap()

    with tile.TileContext(nc) as tc, ExitStack() as ctx:
        const = ctx.enter_context(tc.tile_pool(name="const", bufs=1))
        wqp = ctx.enter_context(tc.tile_pool(name="wqp", bufs=1))
        khp = ctx.enter_context(tc.tile_pool(name="khp", bufs=1))
        vhp = ctx.enter_context(tc.tile_pool(name="vhp", bufs=1))
        xtp = ctx.enter_context(tc.tile_pool(name="xT", bufs=2))
        psA = ctx.enter_context(tc.tile_pool(name="psA", bufs=2, space="PSUM"))
        psP = ctx.enter_context(tc.tile_pool(name="psP", bufs=2, space="PSUM"))
        psUa = ctx.enter_context(tc.tile_pool(name="psUa", bufs=2, space="PSUM"))
        psUb = ctx.enter_context(tc.tile_pool(name="psUb", bufs=2, space="PSUM"))

        ones_col_f32 = const.tile([128, H], F32, tag="ones_col", name="ones_col")
        nc.vector.memset(ones_col_f32[:], 1.0)
        ones_row_f32 = const.tile([1, 128], F32, tag="ones_row_f", name="ones_row_f")
        nc.vector.memset(ones_row_f32[:], 1.0)
        ones_row = const.tile([1, 128], MMD, tag="ones_row", name="ones_row")
        nc.vector.tensor_copy(ones_row[:], ones_row_f32[:])

        # PE warm-up: dependency-free matmuls so the HAM clock gate opens
        # while the first DMAs stream in.
        warm_w_f = const.tile([128, 128], F32, tag="warm_w_f", name="warm_w_f")
        nc.vector.memset(warm_w_f[:], 0.5)
        warm_w = const.tile([128, 128], MMD, tag="warm_w", name="warm_w")
        nc.vector.tensor_copy(warm_w[:], warm_w_f[:])
        warm_x = const.tile([128, 512], MMD, tag="warm_x", name="warm_x")
        for i in range(4):
            nc.vector.tensor_copy(warm_x[:, i * 128:(i + 1) * 128], warm_w_f[:])
        for r in range(64):
            wp = psUa.tile([128, 512], F32, tag="psUa", name="psUa")
            nc.tensor.matmul(wp[:], warm_w[:], warm_x[:], start=True, stop=True)

        # ---- persistent weights (direct cast-DMA loads, no transposes) ----
        WqT = wqp.tile([128, CC, C], MMD, tag="wqt", name="wqt")
        WpT_a = wqp.tile([128, H, C], MMD, tag="wpa", name="wpa")
        WpT_b = [wqp.tile([128, C], MMD, tag=f"wpb{g}", name=f"wpb{g}")
                 for g in range(2)]
        bias_bc = wqp.tile([128, C], F32, tag="bias_bc", name="bias_bc")

        khT_a = [khp.tile([128, N], MMD, tag=f"kha{h}", name=f"kha{h}")
                 for h in range(H)]
        khT_b = [khp.tile([128, N], MMD, tag=f"khb{g}", name=f"khb{g}")
                 for g in range(2)]
        vh = [vhp.tile([128, H, DH + 1], MMD, tag=f"vh{nt}", name=f"vh{nt}")
              for nt in range(KT)]

        def load_wT_grouped(dest, w_dram):
            # dest[p, cc, j] = W.T[cc*128+p, j]
            nc.gpsimd.dma_start(
                dest[:],
                w_dram.rearrange("(cc p) j -> p cc j", p=128))

        def seg_dest(kind, idx, dlo, dhi, a_tiles, b_tiles, col_lo, col_hi):
            t = a_tiles[idx] if kind == "a" else b_tiles[idx]
            return t[dlo:dhi, col_lo:col_hi]

        # ---- phase 1: stage k, v --------------------------------------
        with tc.tile_pool(name="wkv", bufs=1) as wkv:
            WkT = wkv.tile([128, CC, C], MMD, tag="wkt", name="wkt")
            WvT = wkv.tile([128, CC, C], MMD, tag="wvt", name="wvt")
            load_wT_grouped(WkT, wkt_d)
            load_wT_grouped(WvT, wvt_d)

            def load_wq():
                load_wT_grouped(WqT, wqt_d)

            def load_wp_bias():
                # wpt_d is host-packed head-major: rows 0..511 = per-head
                # dd 0..127 (h-major), rows 512..639 / 640..767 = the packed
                # b-tiles (dd 128..191 of heads 0,1 / 2,3).
                nc.gpsimd.dma_start(
                    WpT_a[:],
                    wpt_d[0:512, :].rearrange("(h p) j -> p h j", p=128))
                for g in range(2):
                    nc.gpsimd.dma_start(
                        WpT_b[g][:], wpt_d[512 + g * 128:512 + (g + 1) * 128, :])
                bp_row = wkv.tile([1, C], F32, tag="bp_row", name="bp_row")
                bp_row_r = wkv.tile([1, C], MMD, tag="bp_row_r", name="bp_row_r")
                nc.sync.dma_start(bp_row[:], bp_d[None, :])
                nc.vector.tensor_copy(bp_row_r[:], bp_row[:])
                for jg in range(NJG):
                    ps = psP.tile([128, 512], F32, tag="psP", name="psP")
                    nc.tensor.matmul(ps[:, 0:JGW], ones_row[:],
                                     bp_row_r[:, jg * JGW:(jg + 1) * JGW],
                                     start=True, stop=True)
                    nc.scalar.copy(bias_bc[:, jg * JGW:(jg + 1) * JGW],
                                   ps[:, 0:JGW])

            for ch in range(NCHUNKS):
                n0 = ch * CHUNK
                # -- k chunk: one batched cast-DMA, project to khT ----------
                kTt = xtp.tile([128, CC, CHUNK], MMD, tag="xT", name="kTt")
                nc.gpsimd.dma_start(
                    kTt[:],
                    kt_d[:, n0:n0 + CHUNK].rearrange("(cc p) n -> p cc n", p=128))
                for jc0 in range(0, CC, 2):
                    pss = [psP.tile([128, 512], F32, tag="psP", name="psP")
                           for _ in range(2)]
                    for cc in range(CC):
                        for i in range(2):
                            jc = jc0 + i
                            nc.tensor.matmul(
                                pss[i][:],
                                WkT[:, cc, jc * 128:(jc + 1) * 128],
                                kTt[:, cc, :], start=(cc == 0),
                                stop=(cc == CC - 1))
                    for i in range(2):
                        for (plo, phi, kind, idx, dlo) in _jc_segments(jc0 + i):
                            nc.vector.tensor_copy(
                                seg_dest(kind, idx, dlo, dlo + (phi - plo),
                                         khT_a, khT_b, n0, n0 + CHUNK),
                                pss[i][plo:phi, :])
                # -- v chunk: project to vh natural -------------------------
                vTt = xtp.tile([128, CC, CHUNK], MMD, tag="xT", name="vTt")
                nc.gpsimd.dma_start(
                    vTt[:],
                    vt_d[:, n0:n0 + CHUNK].rearrange("(cc p) n -> p cc n", p=128))
                for ntl in range(4):
                    nt = ch * 4 + ntl
                    pss = [psP.tile([128, 512], F32, tag="psP", name="psP")
                           for _ in range(NJG)]
                    for cc in range(CC):
                        for jg in range(NJG):
                            nc.tensor.matmul(
                                pss[jg][:, 0:JGW],
                                vTt[:, cc, ntl * 128:(ntl + 1) * 128],
                                WvT[:, cc, jg * JGW:(jg + 1) * JGW],
                                start=(cc == 0), stop=(cc == CC - 1))
                    for jg in range(NJG):
                        nc.vector.tensor_copy(
                            vh[nt][:, 2 * jg:2 * jg + 2, 0:DH],
                            pss[jg][:, 0:JGW].rearrange("p (h d) -> p h d", h=2))
                    nc.vector.tensor_copy(
                        vh[nt][:, :, DH:DH + 1],
                        ones_col_f32[:].rearrange("p (h o) -> p h o", h=H))
                if ch == 0:
                    load_wq()
                elif ch == 1:
                    load_wp_bias()

        # ---- phase 2: per q-chunk attention + output projection -----------
        qhp = ctx.enter_context(tc.tile_pool(name="qhp", bufs=1))
        esp = ctx.enter_context(tc.tile_pool(name="esp", bufs=3))
        xop = ctx.enter_context(tc.tile_pool(name="xop", bufs=1))
        scp = ctx.enter_context(tc.tile_pool(name="scp", bufs=2))
        yp = ctx.enter_context(tc.tile_pool(name="yp", bufs=2))

        def q_load(qc):
            n0 = qc * CHUNK
            qTt = xtp.tile([128, CC, CHUNK], MMD, tag="xT", name="qTt")
            nc.gpsimd.dma_start(
                qTt[:],
                qt_d[:, n0:n0 + CHUNK].rearrange("(cc p) n -> p cc n", p=128))
            return qTt

        def q_proj(qc, qTt):
            qhT_a = [qhp.tile([128, CHUNK], MMD, tag=f"qha{h}", name=f"qha{h}")
                     for h in range(H)]
            qhT_b = [qhp.tile([128, CHUNK], MMD, tag=f"qhb{g}", name=f"qhb{g}")
                     for g in range(2)]
            for jc0 in range(0, CC, 2):
                pss = [psP.tile([128, 512], F32, tag="psP", name="psP")
                       for _ in range(2)]
                for cc in range(CC):
                    for i in range(2):
                        jc = jc0 + i
                        nc.tensor.matmul(
                            pss[i][:],
                            WqT[:, cc, jc * 128:(jc + 1) * 128],
                            qTt[:, cc, :], start=(cc == 0), stop=(cc == CC - 1))
                for i in range(2):
                    for (plo, phi, kind, idx, dlo) in _jc_segments(jc0 + i):
                        nc.vector.tensor_copy(
                            seg_dest(kind, idx, dlo, dlo + (phi - plo),
                                     qhT_a, qhT_b, 0, CHUNK),
                            pss[i][plo:phi, :])
            return qhT_a, qhT_b

        def finalize_pre(fu_b):
            # 1-partition RECIPROCAL of the rowsum row. DVE iterative divide
            # (3.4us) - slow but off the PE; ScalarE's table Reciprocal would
            # be 6x faster but evicts the Exp table set (they cannot share the
            # bucket RAM) and the reload thrash costs far more.
            recip = scp.tile([1, CHUNK], MMD, tag="recip", name="recip",
                             bufs=1)
            with nc.allow_low_precision(reason="softmax denom recip f32r"):
                nc.vector.reciprocal(recip[:], fu_b[64:65, :])
            return recip

        def finalize_post(xT_a, xT_b, fh, fu_a, fu_b, recip):
            # broadcast 1/rowsum across partitions (rank-1 ones matmul - the
            # GpSimd partition_broadcast alternative serializes against DVE on
            # the shared SBUF port and slows the whole attention loop down)
            # and normalize during the PSUM evacuation multiplies.
            fblo = (fh % 2) * 64
            bc_ps = psA.tile([128, 512], F32, tag="psA", name="psA")
            nc.tensor.matmul(bc_ps[:], ones_row[:], recip[:],
                             start=True, stop=True)
            bc = scp.tile([128, CHUNK], F32, tag="bc", name="bc", bufs=1)
            nc.scalar.copy(bc[:], bc_ps[:])
            nc.vector.tensor_mul(xT_a[fh][:], fu_a[:], bc[:])
            nc.vector.tensor_mul(xT_b[fh // 2][fblo:fblo + 64, :],
                                 fu_b[0:64, :], bc[0:64, :])

        def attention(qc, qhT_a, qhT_b):
            xT_a = [xop.tile([128, CHUNK], MMD, tag=f"xta{h}", name=f"xta{h}")
                    for h in range(H)]
            xT_b = [xop.tile([128, CHUNK], MMD, tag=f"xtb{g}", name=f"xtb{g}")
                    for g in range(2)]

            def finalize(fh, fu_a, fu_b):
                finalize_post(xT_a, xT_b, fh, fu_a, fu_b, finalize_pre(fu_b))

            pend = None
            for h in range(H):
                blo = (h % 2) * 64
                kb = khT_b[h // 2]
                qb = qhT_b[h // 2]
                u_a = psUa.tile([128, 512], F32, tag="psUa", name="psUa")
                u_b = psUb.tile([65, 512], F32, tag="psUb", name="psUb")
                es_tiles = [None] * KT

                def scores(kt):
                    s = psA.tile([128, 512], F32, tag="psA", name="psA")
                    nc.tensor.matmul(
                        s[:], khT_a[h][:, kt * 128:(kt + 1) * 128],
                        qhT_a[h][:], start=True, stop=False)
                    nc.tensor.matmul(
                        s[:], kb[blo:blo + 64, kt * 128:(kt + 1) * 128],
                        qb[blo:blo + 64, :], start=False, stop=True)
                    es = esp.tile([128, CHUNK], MMD, tag="es", name="es")
                    nc.scalar.activation(es[:], s[:], AF.Exp, scale=SCALE)
                    es_tiles[kt] = es

                def av(kt):
                    es = es_tiles[kt]
                    nc.tensor.matmul(u_a[:], vh[kt][:, h, 0:128], es[:],
                                     start=(kt == 0), stop=(kt == KT - 1))
                    nc.tensor.matmul(u_b[:], vh[kt][:, h, 128:DH + 1], es[:],
                                     start=(kt == 0), stop=(kt == KT - 1))

                scores(0)
                for kt in range(KT - 1):
                    scores(kt + 1)
                    av(kt)
                    if kt == 4 and pend is not None:
                        finalize(*pend)
                        pend = None
                av(KT - 1)
                pend = (h, u_a, u_b)
            # last head: recip starts now; the broadcast + muls are emitted by
            # final_proj between its h0-h2 partial sums so the PE never idles
            # longer than the HAM window.
            recip = finalize_pre(pend[2])
            return xT_a, xT_b, pend, recip

        def final_proj(qc, xT_a, xT_b, pend, recip):
            n0 = qc * CHUNK

            def part_a(pss, ntl):
                # h0..h2 contributions: independent of the pending last-head
                # normalization.
                for h in range(H - 1):
                    blo = (h % 2) * 64
                    for jg in range(NJG):
                        nc.tensor.matmul(
                            pss[jg][:, 0:JGW],
                            xT_a[h][:, ntl * 128:(ntl + 1) * 128],
                            WpT_a[:, h, jg * JGW:(jg + 1) * JGW],
                            start=(h == 0), stop=False)
                    for jg in range(NJG):
                        nc.tensor.matmul(
                            pss[jg][:, 0:JGW],
                            xT_b[h // 2][blo:blo + 64, ntl * 128:(ntl + 1) * 128],
                            WpT_b[h // 2][blo:blo + 64, jg * JGW:(jg + 1) * JGW],
                            start=False, stop=False)

            def part_b(pss, ntl):
                h = H - 1
                blo = (h % 2) * 64
                for jg in range(NJG):
                    nc.tensor.matmul(
                        pss[jg][:, 0:JGW],
                        xT_a[h][:, ntl * 128:(ntl + 1) * 128],
                        WpT_a[:, h, jg * JGW:(jg + 1) * JGW],
                        start=False, stop=False)
                for jg in range(NJG):
                    nc.tensor.matmul(
                        pss[jg][:, 0:JGW],
                        xT_b[h // 2][blo:blo + 64, ntl * 128:(ntl + 1) * 128],
                        WpT_b[h // 2][blo:blo + 64, jg * JGW:(jg + 1) * JGW],
                        start=False, stop=True)

            def evac(pss, ntl):
                ysb = yp.tile([128, C], F32, tag="y", name="y")
                for jg in range(NJG):
                    nc.vector.tensor_add(ysb[:, jg * JGW:(jg + 1) * JGW],
                                         pss[jg][:, 0:JGW],
                                         bias_bc[:, jg * JGW:(jg + 1) * JGW])
                nc.sync.dma_start(
                    y_d[n0 + ntl * 128:n0 + (ntl + 1) * 128, :], ysb[:])

            # groups alternate between the psP and psUa pools (psUa is idle
            # once attention ends) so two groups stay in flight; the 24
            # h0-h2 matmuls of groups 0-1 run while the last head's
            # normalization chain drains.
            def group_tiles(ntl):
                pool, tag = (psP, "psP") if ntl % 2 == 0 else (psUa, "psUa")
                return [pool.tile([128, 512], F32, tag=tag, name=tag)
                        for _ in range(NJG)]

            g0 = group_tiles(0)
            part_a(g0, 0)
            g1 = group_tiles(1)
            part_a(g1, 1)
            finalize_post(xT_a, xT_b, *pend, recip)
            part_b(g0, 0)
            evac(g0, 0)
            part_b(g1, 1)
            evac(g1, 1)
            for ntl in range(2, 4):
                pss = group_tiles(ntl)
                part_a(pss, ntl)
                part_b(pss, ntl)
                evac(pss, ntl)

        # q-chunk pipeline: the next chunk's qT DMA streams during this
        # chunk's attention, and its projection matmuls sit between
        # attention and final_proj as ready PE work that covers the last
        # head's normalization chain.
        qh = q_proj(0, q_load(0))
        for qc in range(NCHUNKS):
            if qc + 1 < NCHUNKS:
                qt_next = q_load(qc + 1)
            xt = attention(qc, *qh)
            if qc + 1 < NCHUNKS:
                qh = q_proj(qc + 1, qt_next)
            final_proj(qc, *xt)

    nc.compile()
    return nc


def _get_built():
    global _BUILT
    if _BUILT is None:
        _BUILT = _build()
    return _BUILT


def run(inputs, trace=False, **kw):
    """Run on all 8 cores; returns (y [B,N,C] float32, BassKernelResults)."""
    from concourse.bass_utils import run_bass_kernel_spmd

    nc = _get_built()
    f32 = np.float32
    wpt = np.asarray(inputs["Wp"], f32).T  # [c', j]
    wpt_packed = np.concatenate(
        [wpt[h * DH:h * DH + 128] for h in range(H)]
        + [wpt[h * DH + 128:(h + 1) * DH] for h in range(H)])
    shared = {
        "WqT": np.ascontiguousarray(np.asarray(inputs["Wq"], f32).T),
        "WkT": np.ascontiguousarray(np.asarray(inputs["Wk"], f32).T),
        "WvT": np.ascontiguousarray(np.asarray(inputs["Wv"], f32).T),
        "WpT": np.ascontiguousarray(wpt_packed),
        "bp": np.ascontiguousarray(np.asarray(inputs["bp"], f32)),
    }
    q = np.asarray(inputs["q"], f32)
    k = np.asarray(inputs["k"], f32)
    v = np.asarray(inputs["v"], f32)
    in_maps = []
    for b in range(B):
        m = dict(shared)
        m["qT"] = np.ascontiguousarray(q[b].T)
        m["kT"] = np.ascontiguousarray(k[b].T)
        m["vT"] = np.ascontiguousarray(v[b].T)
        in_maps.append(m)
    res = run_bass_kernel_spmd(nc, in_maps, list(range(B)), trace=trace, **kw)
    y = np.stack([res.results[b]["y"] for b in range(B)]).astype(np.float32)
    return y, res


def kernel(q, k, v, Wq, Wk, Wv, Wp, bp):
    y, _ = run({"q": q, "k": k, "v": v, "Wq": Wq, "Wk": Wk, "Wv": Wv,
                "Wp": Wp, "bp": bp})
    return y

